# revision 3
# baseline (speedup 1.0000x reference)
"""Trainium2 Bass kernel for nn_NeuroKernel_69956427318000.

Computes, for x [768] and an MLP (2->1024 sigmoid ->128 relu ->1):
    v(i,j) = MLP(x[i], x[j]) for all upper-triangular pairs j >= i
    K = upper-triangular matrix of v (rest zeros)
    return K.T @ K

Strategy (8-core SPMD, single NEFF launch):
  - Column sharding of K: core c owns columns j = 8*t + c, t = 0..95.
    Columns are padded to a uniform per-t length L_t = min(8t+9, 768) so the
    device program is identical on every core (required for SPMD); padded
    entries land strictly below the diagonal and are masked out later.
  - Host gathers x[i]/x[j] per pair into a [74, 2, 512] feed tensor per core.
  - Device: 3-layer MLP fused on-chip, fp32r matmuls (full PE rate at
    near-fp32 accuracy), sigmoid on the scalar engine (the bottleneck),
    scatter of v into a column-major K^T fragment in SBUF.
  - AllGather of the per-core [96, 768] fragments -> permuted K^T; each core
    un-permutes, PE-transposes to K, masks the lower triangle, and computes
    the replicated K^T K. Host returns core 0's output.
"""

import sys

sys.path.insert(0, "/opt/trn_rl_repo")

from contextlib import ExitStack

import numpy as np

import concourse.bass as bass
import concourse.mybir as mybir
import concourse.tile as tile
from concourse import bacc, bass_utils
from concourse.masks import make_identity

N = 768
NCORES = 8
TCOLS = 96  # columns per core
CHUNK = 512  # pairs per matmul N-chunk
NCHUNKS = 74  # chunks per core (padded)
NSB = NCHUNKS // 2  # super-blocks of 2 chunks (1024 pairs)
NTILES = N // 128  # 6

F32 = mybir.dt.float32
F32R = mybir.dt.float32r

# Per-t padded column lengths and flat offsets (identical on every core).
_L = [min(8 * t + 9, N) for t in range(TCOLS)]
_F = np.concatenate([[0], np.cumsum(_L)])  # _F[t] = flat start of column t
P_CORE = int(_F[-1])  # 37343 real pairs; padded to NCHUNKS*CHUNK = 37888

# Scatter segments: chunk k's v values [src, src+ln) go to CT[t, dst:dst+ln].
_SEGS = [[] for _ in range(NCHUNKS)]
for _t in range(TCOLS):
    _s, _e = int(_F[_t]), int(_F[_t] + _L[_t])
    while _s < _e:
        _k = _s // CHUNK
        _take = min(_e, (_k + 1) * CHUNK) - _s
        _SEGS[_k].append((_s - _k * CHUNK, _t, _s - int(_F[_t]), _take))
        _s += _take


def build_module():
    nc = bacc.Bacc(
        "TRN2", target_bir_lowering=False, debug=False, num_devices=NCORES
    )
    pairs_d = nc.dram_tensor(
        "pairs", [NCHUNKS, 2, CHUNK], F32R, kind="ExternalInput"
    ).ap()
    w1t_d = nc.dram_tensor("w1t", [2, 1024], F32R, kind="ExternalInput").ap()
    w2t_d = nc.dram_tensor("w2t", [1024, 128], F32R, kind="ExternalInput").ap()
    w3t_d = nc.dram_tensor("w3t", [128, 1], F32R, kind="ExternalInput").ap()
    b1r_d = nc.dram_tensor("b1r", [128, 8], F32, kind="ExternalInput").ap()
    b2r_d = nc.dram_tensor("b2r", [128, 1], F32, kind="ExternalInput").ap()
    b3r_d = nc.dram_tensor("b3r", [1, 1], F32, kind="ExternalInput").ap()
    out_d = nc.dram_tensor("out", [N, N], F32, kind="ExternalOutput").ap()

    with tile.TileContext(nc) as tc:
        with (
            tc.tile_pool(name="const", bufs=1) as const,
            tc.tile_pool(name="rhsp", bufs=3) as rhsp,
            tc.tile_pool(name="h1p", bufs=4) as h1p,
            tc.tile_pool(name="h2sp", bufs=3) as h2sp,
            tc.tile_pool(name="vbp", bufs=3) as vbp,
            tc.tile_pool(name="dram", bufs=1, space="DRAM") as dram,
        ):
            # --- load weights / biases ---
            w1s = const.tile([2, 1024], F32R, name="w1s")
            w2s = const.tile([128, 1024], F32R, name="w2s")
            w3s = const.tile([128, 1], F32R, name="w3s")
            b1s = const.tile([128, 8], F32, name="b1s")
            b2s = const.tile([128, 1], F32, name="b2s")
            b3s = const.tile([1, 1], F32, name="b3s")
            ct = const.tile([TCOLS, N], F32, name="ct")

            nc.sync.dma_start(w1s[:], w1t_d[:])
            for k in range(8):
                nc.sync.dma_start(
                    w2s[:, 128 * k : 128 * (k + 1)],
                    w2t_d[128 * k : 128 * (k + 1), :],
                )
            nc.sync.dma_start(w3s[:], w3t_d[:])
            nc.sync.dma_start(b1s[:], b1r_d[:])
            nc.sync.dma_start(b2s[:], b2r_d[:])
            nc.sync.dma_start(b3s[:], b3r_d[:])
            nc.vector.memset(ct[:], 0.0)

            # --- main MLP loop over super-blocks of 1024 pairs ---
            main_psum = ExitStack()
            prep = main_psum.enter_context(
                tc.tile_pool(name="prep", bufs=2, space="PSUM")
            )
            h2pp = main_psum.enter_context(
                tc.tile_pool(name="h2pp", bufs=2, space="PSUM")
            )
            vpp = main_psum.enter_context(
                tc.tile_pool(name="vpp", bufs=2, space="PSUM")
            )
            for s in range(NSB):
                rhs = rhsp.tile([2, 1024], F32R, name="rhs")
                nc.sync.dma_start(rhs[:, 0:CHUNK], pairs_d[2 * s, :, :])
                nc.sync.dma_start(rhs[:, CHUNK:], pairs_d[2 * s + 1, :, :])

                h2ps = [
                    h2pp.tile([128, CHUNK], F32, name="h2ps") for _ in range(2)
                ]
                for f in range(8):
                    pre = prep.tile([128, 1024], F32, name="pre")
                    for t in range(2):
                        nc.tensor.matmul(
                            pre[:, CHUNK * t : CHUNK * (t + 1)],
                            w1s[:, 128 * f : 128 * (f + 1)],
                            rhs[:, CHUNK * t : CHUNK * (t + 1)],
                            start=True,
                            stop=True,
                        )
                    h1 = h1p.tile([128, 1024], F32R, name="h1")
                    nc.scalar.activation(
                        h1[:],
                        pre[:],
                        mybir.ActivationFunctionType.Sigmoid,
                        bias=b1s[:, f : f + 1],
                        scale=1.0,
                    )
                    for t in range(2):
                        nc.tensor.matmul(
                            h2ps[t][:],
                            w2s[:, 128 * f : 128 * (f + 1)],
                            h1[:, CHUNK * t : CHUNK * (t + 1)],
                            start=(f == 0),
                            stop=(f == 7),
                        )

                for t in range(2):
                    k = 2 * s + t
                    h2s = h2sp.tile([128, CHUNK], F32R, name="h2s")
                    nc.vector.tensor_scalar(
                        h2s[:],
                        h2ps[t][:],
                        b2s[:],
                        0.0,
                        op0=mybir.AluOpType.add,
                        op1=mybir.AluOpType.max,
                    )
                    v = vpp.tile([1, CHUNK], F32, name="v")
                    nc.tensor.matmul(
                        v[:], w3s[:], h2s[:], start=True, stop=True
                    )
                    vb = vbp.tile([1, CHUNK], F32, name="vb")
                    nc.vector.tensor_scalar(
                        vb[:], v[:], b3s[:], None, op0=mybir.AluOpType.add
                    )
                    for src, t_col, dst, ln in _SEGS[k]:
                        nc.sync.dma_start(
                            ct[t_col : t_col + 1, dst : dst + ln],
                            vb[:, src : src + ln],
                        )

            main_psum.close()

            # --- exchange: AllGather the [96, 768] fragments ---
            ct_dram = dram.tile([TCOLS, N], F32)
            ct_all = dram.tile([NCORES * TCOLS, N], F32, addr_space="Shared")
            nc.sync.dma_start(ct_dram[:], ct[:])
            nc.gpsimd.collective_compute(
                "AllGather",
                mybir.AluOpType.bypass,
                replica_groups=[list(range(NCORES))],
                ins=[ct_dram.opt()],
                outs=[ct_all.opt()],
            )

            # --- un-permute into KT tiles: ktm[it][p, i] = K[i, 128*it + p] ---
            ktms = [
                const.tile([128, N], F32, name=f"ktm{i}") for i in range(NTILES)
            ]
            for it in range(NTILES):
                view = ktms[it][:].rearrange("(q c) f -> c q f", c=8)
                for c in range(8):
                    nc.sync.dma_start(
                        view[c],
                        ct_all[96 * c + 16 * it : 96 * c + 16 * it + 16, :],
                    )

            # --- transpose to K tiles, mask lower triangle ---
            ident = const.tile([128, 128], F32, name="ident")
            make_identity(nc, ident[:])
            kss = [
                const.tile([128, N], F32, name=f"ks{i}") for i in range(NTILES)
            ]
            for it in range(NTILES):
                nc.vector.memset(kss[it][:], 0.0)
            with tc.tile_pool(name="tpp", bufs=2, space="PSUM") as tpp:
                for jt in range(NTILES):
                    for it in range(jt + 1):
                        tp = tpp.tile([128, 128], F32, name="tp")
                        nc.tensor.transpose(
                            tp[:], ktms[jt][:, 128 * it : 128 * (it + 1)], ident[:]
                        )
                        nc.vector.tensor_copy(
                            kss[it][:, 128 * jt : 128 * (jt + 1)], tp[:]
                        )
                for it in range(NTILES):
                    nc.gpsimd.affine_select(
                        out=kss[it][:, 128 * it : 128 * (it + 1)],
                        in_=kss[it][:, 128 * it : 128 * (it + 1)],
                        compare_op=mybir.AluOpType.is_ge,
                        fill=0.0,
                        base=0,
                        pattern=[[1, 128]],
                        channel_multiplier=-1,
                    )

                # --- C = K^T K (fp32, replicated) ---
                NB = 384
                with (
                    tc.tile_pool(name="cpp", bufs=2, space="PSUM") as cpp,
                    tc.tile_pool(name="csb", bufs=2) as csb,
                ):
                    for mi in range(NTILES):
                        for nb in range(2):
                            cps = cpp.tile([128, NB], F32, name="cps")
                            for ki in range(mi + 1):
                                nc.tensor.matmul(
                                    cps[:],
                                    kss[ki][:, 128 * mi : 128 * (mi + 1)],
                                    kss[ki][:, NB * nb : NB * (nb + 1)],
                                    start=(ki == 0),
                                    stop=(ki == mi),
                                )
                            cs = csb.tile([128, NB], F32, name="cs")
                            nc.vector.tensor_copy(cs[:], cps[:])
                            nc.sync.dma_start(
                                out_d[
                                    128 * mi : 128 * (mi + 1),
                                    NB * nb : NB * (nb + 1),
                                ],
                                cs[:],
                            )
    nc.compile()
    return nc


_CACHED = None


def _get_module():
    global _CACHED
    if _CACHED is None:
        _CACHED = build_module()
    return _CACHED


def _host_inputs(x, W1, b1, W2, b2, W3, b3):
    x = np.asarray(x, dtype=np.float32)
    w1t = np.ascontiguousarray(np.asarray(W1, np.float32).T)  # [2, 1024]
    w2t = np.ascontiguousarray(np.asarray(W2, np.float32).T)  # [1024, 128]
    w3t = np.ascontiguousarray(np.asarray(W3, np.float32).T)  # [128, 1]
    b1r = np.ascontiguousarray(np.asarray(b1, np.float32).reshape(8, 128).T)
    b2r = np.asarray(b2, np.float32).reshape(128, 1)
    b3r = np.asarray(b3, np.float32).reshape(1, 1)

    ii = np.concatenate([np.arange(_L[t]) for t in range(TCOLS)])
    jj_base = np.concatenate(
        [np.full(_L[t], 8 * t, dtype=np.int64) for t in range(TCOLS)]
    )
    pad = NCHUNKS * CHUNK - P_CORE
    ii = np.concatenate([ii, np.zeros(pad, dtype=np.int64)])
    jj_base = np.concatenate([jj_base, np.zeros(pad, dtype=np.int64)])

    in_maps = []
    for c in range(NCORES):
        jj = np.minimum(jj_base + c, N - 1)
        xi = x[ii].reshape(NCHUNKS, CHUNK)
        xj = x[jj].reshape(NCHUNKS, CHUNK)
        pairs = np.ascontiguousarray(
            np.stack([xi, xj], axis=1), dtype=np.float32
        )
        in_maps.append(
            {
                "pairs": pairs,
                "w1t": w1t,
                "w2t": w2t,
                "w3t": w3t,
                "b1r": b1r,
                "b2r": b2r,
                "b3r": b3r,
            }
        )
    return in_maps


def run(x, W1, b1, W2, b2, W3, b3, trace=False, **trace_kwargs):
    nc = _get_module()
    in_maps = _host_inputs(x, W1, b1, W2, b2, W3, b3)
    res = bass_utils.run_bass_kernel_spmd(
        nc, in_maps, core_ids=list(range(NCORES)), trace=trace, **trace_kwargs
    )
    return np.asarray(res.results[0]["out"], dtype=np.float32), res


def kernel(x, W1, b1, W2, b2, W3, b3):
    out, _ = run(x, W1, b1, W2, b2, W3, b3)
    return out
